# revision 1
# baseline (speedup 1.0000x reference)
"""DeepSATConv GNN message-passing kernel for 8 Trainium2 NeuronCores.

Math note: the reference computes a per-channel segment-softmax over
msg = self_h[src] + neib_h[dst].  Within a dst-segment, neib_h[dst] (and
b_self, b_nb) are constant per channel, so they cancel in the softmax.
Hence alpha = segsoftmax(h[src] @ W_self.T) exactly, and
out[n] = segsum(e * h[src]) / segsum(e)  with e = exp((h @ W_self.T)[src]),
falling back to h[n] for zero-in-degree nodes.  W_nb / b_nb / b_self do
not affect the output at all.

Sharding: nodes are split across the 8 cores (2500 each); edges are
partitioned by destination node so segment reductions stay core-local;
h is replicated (the "halo gather" degenerates to replication).

Per core the kernel
  A) computes Z = h @ [W_self.T | I] = [self_h | h] for all nodes into
     core-local HBM (replicated compute; cheaper than collectives, and
     packing h alongside self_h lets one dma_gather descriptor fetch
     both operands per edge — SWDGE descriptor generation on the Q7 is
     the dominant cost of gathers),
  B) for each 128-node tile, dma_gathers Z[src] for the tile's
     (dst-sorted, padded) edge list, then for each 128-edge chunk
     builds a one-hot selector S[e, n] = (dst_local[e] == n) on the DVE
     and accumulates  [denom | numer] = S.T @ [exp(sh) | exp(sh) * hs]
     into a PSUM bank over all chunks of the tile,
  C) finalizes out = numer / max(denom, tiny), with copy_predicated
     restoring h for empty nodes, and writes the tile to HBM.
"""

import os
import numpy as np

N_NODES = 20000
N_EDGES = 320000
D = 256
CORES = 8
NPC = N_NODES // CORES          # 2500 nodes per core
NT = (NPC + 127) // 128         # 20 node tiles per core
NROWS = NT * 128                # 2560 padded rows per core
NT_ALL = 160                    # phase-A tiles (two 80-tile Z blocks)
NPAD = NT_ALL * 128             # 20480
NPB = 2                         # Z source blocks (phase A/B overlap)
NBH = NPAD // NPB               # rows per Z block
BB = 6                          # chunks per exp/mult batch

# float32r runs the selector matmul at 4x the fp32 rate but rounds
# operands to ~tf32 precision (~8e-4 output error vs ~3e-5 for fp32).
USE_F32R = os.environ.get("GNN_F32R", "0") == "1"

_cache = {}


def _build(caps):
    import concourse.bacc as bacc
    import concourse.mybir as mybir
    from concourse.tile import TileContext

    nc = bacc.Bacc("TRN2")
    f32 = mybir.dt.float32
    mm_dt = mybir.dt.float32r if USE_F32R else f32

    bf16 = mybir.dt.bfloat16
    NCH = sum(sum(r) for r in caps)     # total chunks across tiles/blocks
    NIX = 128 * NCH                     # total gathered edge slots
    hT_d = nc.dram_tensor("hT", [128, 2, 2, NPAD], bf16, kind="ExternalInput")
    WI_d = nc.dram_tensor("WI", [128, 2, 2, 2 * D], bf16, kind="ExternalInput")
    idx_d = nc.dram_tensor("idx", [128, NIX // 16], mybir.dt.int16, kind="ExternalInput")
    S_d = nc.dram_tensor("S", [128, NCH, 128], f32, kind="ExternalInput")
    hown_d = nc.dram_tensor("hown", [NROWS, D], f32, kind="ExternalInput")
    out_d = nc.dram_tensor("out", [NROWS, D], f32, kind="ExternalOutput")

    CMAX = max(a + b for a, b in caps)
    with TileContext(nc) as tc:
        with (
            tc.tile_pool(name="const", bufs=1) as constp,
            tc.tile_pool(name="pha", bufs=3) as pha,
            tc.tile_pool(name="gat", bufs=2) as gat,
            tc.tile_pool(name="wrk", bufs=6) as wrk,
            tc.tile_pool(name="fin", bufs=2) as fin,
            tc.tile_pool(name="psa", bufs=2, space="PSUM") as psa,
            tc.tile_pool(name="psb", bufs=3, space="PSUM") as psb,
            tc.tile_pool(name="dram", bufs=1, space="DRAM") as dramp,
        ):
            z_blk = []
            for s_ in range(NPB):
                zb = dramp.tile([NBH, 2 * D], f32, tag=f"zblk{s_}")
                z_blk.append(zb)

            # ---- phase A: Z = h @ [W_self.T | I] = [self_h | h], all nodes ----
            # bf16 hi/lo split: h = hi + lo, W.T columns split likewise into
            # WI_hi = [W_hi.T | I], WI_lo = [W_lo.T | 0]; three bf16 products
            # hi@WI_hi + hi@WI_lo + lo@WI_hi reproduce fp32 to ~1e-5.
            WI_sb = constp.tile([128, 2, 2, 2 * D], bf16)
            nc.sync.dma_start(WI_sb[:, :, :, :], WI_d[:, :, :, :])
            for i in range(NT_ALL):
                hT_sb = pha.tile([128, 2, 2, 128], bf16, tag="hT")
                nc.sync.dma_start(hT_sb[:, :, :, :], hT_d[:, :, :, i * 128:(i + 1) * 128])
                ps = psa.tile([128, 2 * D], f32, tag="ps")
                nmm = 0
                for hw, ww in ((0, 0), (0, 1), (1, 0)):
                    for kb in range(2):
                        nc.tensor.matmul(
                            ps[:, :], hT_sb[:, hw, kb, :], WI_sb[:, ww, kb, :],
                            start=(nmm == 0), stop=(nmm == 5),
                        )
                        nmm += 1
                z_sb = pha.tile([128, 2 * D], f32, tag="zs")
                nc.scalar.copy(z_sb[:, :], ps[:, :])
                blk, row = divmod(i * 128, NBH)
                nc.sync.dma_start(z_blk[blk][row:row + 128, :], z_sb[:, :])

            # ---- constants ----
            idx_sb = constp.tile([128, NIX // 16], mybir.dt.int16)
            nc.sync.dma_start(idx_sb[:, :], idx_d[:, :])

            # ---- phase B: per node-tile segment softmax ----
            chunk_off = 0   # global chunk counter (indexes idx/S/dstl layout)
            for t in range(NT):
                zx_t = gat.tile([128, CMAX, 2 * D], f32, tag="zx")
                C_t = caps[t][0] + caps[t][1]
                zoff = 0
                for s_ in range(NPB):
                    Cs = caps[t][s_]
                    if Cs == 0:
                        continue
                    CAPs = 128 * Cs
                    io = (chunk_off + zoff) * 8
                    nc.gpsimd.dma_gather(
                        zx_t[:, zoff:zoff + Cs, :], z_blk[s_][:, :],
                        idx_sb[:, io:io + 8 * Cs], CAPs, CAPs, 2 * D,
                        single_packet=False,
                    )
                    zoff += Cs
                acc = psb.tile([128, 2 * D], f32, tag="acc")
                for g in range((C_t + BB - 1) // BB):
                    b = min(BB, C_t - g * BB)
                    eX = wrk.tile([128, BB, 2 * D], mm_dt, tag="eX")
                    Sg = wrk.tile([128, BB, 128], f32, tag="Sg")
                    so = chunk_off + g * BB
                    nc.sync.dma_start(Sg[:, 0:b, :], S_d[:, so:so + b, :])
                    nc.scalar.activation(
                        eX[:, 0:b, 0:D], zx_t[:, g * BB:g * BB + b, 0:D],
                        mybir.ActivationFunctionType.Exp,
                    )
                    nc.vector.tensor_tensor(
                        eX[:, 0:b, D:2 * D], eX[:, 0:b, 0:D],
                        zx_t[:, g * BB:g * BB + b, D:2 * D],
                        mybir.AluOpType.mult,
                    )
                    for j in range(b):
                        k = g * BB + j
                        nc.tensor.matmul(
                            acc[:, :], Sg[:, j, :], eX[:, j, :],
                            start=(k == 0), stop=(k == C_t - 1),
                        )
                chunk_off += C_t

                # ---- finalize tile ----
                accs = fin.tile([128, 2 * D], f32, tag="accs")
                nc.scalar.copy(accs[:, :], acc[:, :])
                dmax = fin.tile([128, D], f32, tag="dmax")
                nc.vector.tensor_scalar(
                    dmax[:, :], accs[:, 0:D], 1e-37, None, mybir.AluOpType.max
                )
                rec = fin.tile([128, D], f32, tag="rec")
                nc.vector.reciprocal(rec[:, :], dmax[:, :])
                res = fin.tile([128, D], f32, tag="res")
                nc.vector.tensor_tensor(
                    res[:, :], accs[:, D:2 * D], rec[:, :], mybir.AluOpType.mult
                )
                mask = fin.tile([128, D], mybir.dt.uint8, tag="mask")
                nc.vector.tensor_scalar(
                    mask[:, :], accs[:, 0:D], 0.0, None, mybir.AluOpType.is_equal
                )
                hown_sb = fin.tile([128, D], f32, tag="hown")
                nc.sync.dma_start(hown_sb[:, :], hown_d[t * 128:(t + 1) * 128, :])
                nc.vector.copy_predicated(res[:, :], mask[:, :], hown_sb[:, :])
                nc.sync.dma_start(out_d[t * 128:(t + 1) * 128, :], res[:, :])
    nc.compile()
    return nc


def _wrap_idx(ix):
    # dma_gather index layout: logical index i lands at output
    # [partition i%128, slot i//128]; the SBUF index tile stores it at
    # [i%16, 8*(i//128) + (i%128)//16], replicated over the 8 Q7 cores.
    w = ix.astype(np.int16).reshape(-1, 8, 16).transpose(2, 0, 1).reshape(16, -1)
    return np.tile(w, (8, 1))


def kernel(h, W_nb, b_nb, W_self, b_self, src, dst):
    from concourse.bass_utils import run_bass_kernel_spmd

    h = np.ascontiguousarray(np.asarray(h, dtype=np.float32))
    W = np.asarray(W_self, dtype=np.float32)
    src = np.asarray(src, dtype=np.int64)
    dst = np.asarray(dst, dtype=np.int64)

    order = np.argsort(dst, kind="stable")
    src_s = src[order]
    dst_s = dst[order]

    # per-(core, tile) edge ranges; tiles are 128 consecutive owned nodes
    tile_base = []
    for c in range(CORES):
        for t in range(NT):
            tile_base.append(c * NPC + t * 128)
    bounds_lo = np.searchsorted(dst_s, np.array(tile_base), side="left")
    hi_nodes = [min(b + 128, (b // NPC + 1) * NPC) for b in tile_base]
    bounds_hi = np.searchsorted(dst_s, np.array(hi_nodes), side="left")

    # split each tile's edges by src block; caps shared across cores (SPMD)
    per_ct = {}
    cnt = np.zeros((CORES, NT, NPB), dtype=np.int64)
    for c in range(CORES):
        for t in range(NT):
            i = c * NT + t
            lo, hi = int(bounds_lo[i]), int(bounds_hi[i])
            blk = src_s[lo:hi] // NBH
            for s_ in range(NPB):
                sel = np.nonzero(blk == s_)[0]
                per_ct[(c, t, s_)] = (src_s[lo:hi][sel], dst_s[lo:hi][sel] - tile_base[i])
                cnt[c, t, s_] = len(sel)
    caps = [
        [int((cnt[:, t, s_].max() + 127) // 128) for s_ in range(NPB)]
        for t in range(NT)
    ]
    assert max(a + b for a, b in caps) <= 36, f"edge distribution too skewed: {caps}"
    NCH = sum(sum(r) for r in caps)

    # host-side layout prep: bf16 hi/lo split of h and W for phase A
    import ml_dtypes
    bf = ml_dtypes.bfloat16
    h_hi = h.astype(bf)
    h_lo = (h - h_hi.astype(np.float32)).astype(bf)
    W_hi = W.astype(bf)
    W_lo = (W - W_hi.astype(np.float32)).astype(bf)

    hT = np.zeros((2, D, NPAD), dtype=bf)
    hT[0, :, :N_NODES] = h_hi.T
    hT[1, :, :N_NODES] = h_lo.T
    hT = np.ascontiguousarray(
        hT.reshape(2, 2, 128, NPAD).transpose(2, 0, 1, 3)
    )
    WI = np.zeros((2, D, 2 * D), dtype=bf)
    WI[0, :, :D] = W_hi.T
    WI[1, :, :D] = W_lo.T
    WI[0, np.arange(D), D + np.arange(D)] = bf(1.0)
    WI = np.ascontiguousarray(
        WI.reshape(2, 2, 128, 2 * D).transpose(2, 0, 1, 3)
    )

    in_maps = []
    for c in range(CORES):
        idx_parts = []
        S_all = np.zeros((128, NCH, 128), dtype=np.float32)
        coff = 0
        for t in range(NT):
            for s_ in range(NPB):
                Cs = caps[t][s_]
                if Cs == 0:
                    continue
                CAPs = 128 * Cs
                ss, dl_real = per_ct[(c, t, s_)]
                n = len(ss)
                spad = np.zeros(CAPs, dtype=np.int64)
                spad[:n] = ss - s_ * NBH      # block-local row index
                dl = np.full(CAPs, -1, dtype=np.int64)
                dl[:n] = dl_real
                idx_parts.append(_wrap_idx(spad))
                ei = np.nonzero(dl >= 0)[0]
                S_all[ei % 128, coff + ei // 128, dl[ei]] = 1.0
                coff += Cs
        hown = np.zeros((NROWS, D), dtype=np.float32)
        hown[:NPC] = h[c * NPC:(c + 1) * NPC]
        in_maps.append({
            "hT": hT,
            "WI": WI,
            "idx": np.ascontiguousarray(np.concatenate(idx_parts, axis=1)),
            "S": S_all,
            "hown": hown,
        })

    key = tuple(tuple(r) for r in caps)
    if key not in _cache:
        _cache[key] = _build(caps)
    nc = _cache[key]

    res = run_bass_kernel_spmd(nc, in_maps, core_ids=list(range(CORES)))
    out = np.concatenate(
        [res.results[c]["out"][:NPC] for c in range(CORES)], axis=0
    )
    return out.astype(np.float32)



# revision 4
# speedup vs baseline: 1.1434x; 1.1434x over previous
"""DeepSATConv GNN message-passing kernel for 8 Trainium2 NeuronCores.

Math note: the reference computes a per-channel segment-softmax over
msg = self_h[src] + neib_h[dst].  Within a dst-segment, neib_h[dst] (and
b_self, b_nb) are constant per channel, so they cancel in the softmax.
Hence alpha = segsoftmax((h @ W_self.T)[src]) exactly, and
out[n] = segsum(e * h[src]) / segsum(e)  with e = exp((h @ W_self.T)[src]),
falling back to h[n] for zero-in-degree nodes (patched host-side).

Sharding: nodes split across 8 cores (2500 each); edges partitioned by
destination so segment reductions stay core-local; h replicated.

Per core:
  A) Z = [Q | P] with Q = exp(h @ W_self.T), P = Q * h, for ALL nodes,
     in bf16, written to core-local DRAM in two src blocks (30%/70%)
     so phase-B gathers of block 0 can start while block 1 computes.
  B) per 128-dst-node tile, dma_gather Z[src] for the tile's
     (dst-sorted, padded) edge slots; for each 128-edge chunk build the
     one-hot selector S[e, n] = (dst_local[e] == n) on the DVE
     (is_equal vs an iota row) and accumulate
     [denom | numer] = S.T @ [Q | P] into a PSUM bank (bf16 matmuls).
     All block-0 gathers are emitted before any block-1 work so the Q7
     descriptor-generation stream (the serial bottleneck, ~7ns/row)
     never stalls on the in-order phase-A dependency.
  C) out = numer / max(denom, tiny); zero-degree rows are fixed on host.
"""

import numpy as np

N_NODES = 20000
N_EDGES = 320000
D = 256
CORES = 8
NPC = N_NODES // CORES          # 2500 nodes per core
NT = (NPC + 127) // 128         # 20 dst tiles per core
NROWS = NT * 128                # 2560 padded rows per core
NT_ALL = 160                    # phase-A tiles
NPAD = NT_ALL * 128             # 20480
NPB = 2
NT_B0 = 48                      # phase-A tiles in src block 0 (30%)
NBH0 = NT_B0 * 128              # 6144 rows
NBH1 = NPAD - NBH0              # 14336 rows

_cache = {}


def _build(caps):
    import concourse.bacc as bacc
    import concourse.mybir as mybir
    from concourse.tile import TileContext

    nc = bacc.Bacc("TRN2")
    f32 = mybir.dt.float32
    bf16 = mybir.dt.bfloat16

    NCH = sum(sum(r) for r in caps)     # total chunks across tiles/blocks
    NIX = 128 * NCH                     # total gathered edge slots
    CMAX = [max(c[s] for c in caps) for s in range(NPB)]

    hT_d = nc.dram_tensor("hT", [128, NT_ALL, 2, 128], bf16, kind="ExternalInput")
    hrow_d = nc.dram_tensor("hrow", [NPAD, D], bf16, kind="ExternalInput")
    WT_d = nc.dram_tensor("WT", [128, 2, D], bf16, kind="ExternalInput")
    iota_d = nc.dram_tensor("iota", [128, 128], bf16, kind="ExternalInput")
    dstl_d = nc.dram_tensor("dstl", [128, NCH], f32, kind="ExternalInput")
    idx_d = nc.dram_tensor("idx", [128, NIX // 16], mybir.dt.int16, kind="ExternalInput")
    out_d = nc.dram_tensor("out", [NROWS, D], f32, kind="ExternalOutput")

    with TileContext(nc) as tc:
        with (
            tc.tile_pool(name="const", bufs=1) as constp,
            tc.tile_pool(name="pha", bufs=3) as pha,
            tc.tile_pool(name="gat0", bufs=NT) as gat0,
            tc.tile_pool(name="gat1", bufs=2) as gat1,
            tc.tile_pool(name="swk", bufs=4) as swk,
            tc.tile_pool(name="fin", bufs=2) as fin,
            tc.tile_pool(name="psa", bufs=3, space="PSUM") as psa,
            tc.tile_pool(name="psb", bufs=3, space="PSUM") as psb,
            tc.tile_pool(name="dram", bufs=1, space="DRAM") as dramp,
        ):
            z_blk = []
            for s_ in range(NPB):
                zb = dramp.tile([NBH0 if s_ == 0 else NBH1, 2 * D], bf16, tag=f"zblk{s_}")
                z_blk.append(zb)

            WT_sb = constp.tile([128, 2, D], bf16)
            nc.sync.dma_start(WT_sb[:, :, :], WT_d[:, :, :])
            iota_sb = constp.tile([128, 128], bf16)
            nc.sync.dma_start(iota_sb[:, :], iota_d[:, :])
            dstl_sb = constp.tile([128, NCH], f32)
            nc.sync.dma_start(dstl_sb[:, :], dstl_d[:, :])
            idx_sb = constp.tile([128, NIX // 16], mybir.dt.int16)
            nc.sync.dma_start(idx_sb[:, :], idx_d[:, :])

            # ---- phase A: Z = [exp(h @ W.T) | exp(h @ W.T) * h], all nodes ----
            for i in range(NT_ALL):
                hT_sb = pha.tile([128, 2, 128], bf16, tag="hT")
                nc.sync.dma_start(hT_sb[:, :, :], hT_d[:, i, :, :])
                ps = psa.tile([128, D], f32, tag="ps")
                for kb in range(2):
                    nc.tensor.matmul(
                        ps[:, :], hT_sb[:, kb, :], WT_sb[:, kb, :],
                        start=(kb == 0), stop=(kb == 1),
                    )
                z_sb = pha.tile([128, 2 * D], bf16, tag="zs")
                nc.scalar.activation(
                    z_sb[:, 0:D], ps[:, :], mybir.ActivationFunctionType.Exp
                )
                hr_sb = pha.tile([128, D], bf16, tag="hr")
                nc.sync.dma_start(hr_sb[:, :], hrow_d[i * 128:(i + 1) * 128, :])
                nc.vector.tensor_tensor(
                    z_sb[:, D:2 * D], z_sb[:, 0:D], hr_sb[:, :],
                    mybir.AluOpType.mult,
                )
                if i < NT_B0:
                    nc.sync.dma_start(z_blk[0][i * 128:(i + 1) * 128, :], z_sb[:, :])
                else:
                    row = (i - NT_B0) * 128
                    nc.sync.dma_start(z_blk[1][row:row + 128, :], z_sb[:, :])

            # chunk offsets in host layout: all (t, s=0) regions, then (t, s=1)
            off0 = [0] * NT
            off1 = [0] * NT
            o = 0
            for t in range(NT):
                off0[t] = o
                o += caps[t][0]
            for t in range(NT):
                off1[t] = o
                o += caps[t][1]

            # ---- phase B head: all block-0 gathers (Q7 stream never stalls) ----
            zx0 = []
            for t in range(NT):
                zt = gat0.tile([128, CMAX[0], 2 * D], bf16, tag="zx0")
                Cs = caps[t][0]
                io = off0[t] * 8
                nc.gpsimd.dma_gather(
                    zt[:, 0:Cs, :], z_blk[0][:, :],
                    idx_sb[:, io:io + 8 * Cs], 128 * Cs, 128 * Cs, 2 * D,
                    single_packet=False,
                )
                zx0.append(zt)

            # ---- phase B: per-tile block-1 gather, selector matmuls, finalize ----
            for t in range(NT):
                C0, C1 = caps[t][0], caps[t][1]
                zx1 = gat1.tile([128, CMAX[1], 2 * D], bf16, tag="zx1")
                io = off1[t] * 8
                nc.gpsimd.dma_gather(
                    zx1[:, 0:C1, :], z_blk[1][:, :],
                    idx_sb[:, io:io + 8 * C1], 128 * C1, 128 * C1, 2 * D,
                    single_packet=False,
                )
                acc = psb.tile([128, 2 * D], f32, tag="acc")
                for j in range(C0 + C1):
                    co = (off0[t] + j) if j < C0 else (off1[t] + j - C0)
                    Sg = swk.tile([128, 128], bf16, tag="Sg")
                    nc.vector.tensor_scalar(
                        Sg[:, :], iota_sb[:, :], dstl_sb[:, co:co + 1], None,
                        mybir.AluOpType.is_equal,
                    )
                    src_t = zx0[t] if j < C0 else zx1
                    jj = j if j < C0 else j - C0
                    nc.tensor.matmul(
                        acc[:, :], Sg[:, :], src_t[:, jj, :],
                        start=(j == 0), stop=(j == C0 + C1 - 1),
                    )

                # ---- finalize tile ----
                accs = fin.tile([128, 2 * D], f32, tag="accs")
                nc.scalar.copy(accs[:, :], acc[:, :])
                dmax = fin.tile([128, D], f32, tag="dmax")
                nc.vector.tensor_scalar(
                    dmax[:, :], accs[:, 0:D], 1e-37, None, mybir.AluOpType.max
                )
                rec = fin.tile([128, D], f32, tag="rec")
                nc.vector.reciprocal(rec[:, :], dmax[:, :])
                res = fin.tile([128, D], f32, tag="res")
                nc.vector.tensor_tensor(
                    res[:, :], accs[:, D:2 * D], rec[:, :], mybir.AluOpType.mult
                )
                nc.sync.dma_start(out_d[t * 128:(t + 1) * 128, :], res[:, :])
    nc.compile()
    return nc


def _wrap_idx(ix):
    # dma_gather index layout: logical index i lands at output
    # [partition i%128, slot i//128]; the SBUF index tile stores it at
    # [i%16, 8*(i//128) + (i%128)//16], replicated over the 8 Q7 cores.
    w = ix.astype(np.int16).reshape(-1, 8, 16).transpose(2, 0, 1).reshape(16, -1)
    return np.tile(w, (8, 1))


def kernel(h, W_nb, b_nb, W_self, b_self, src, dst):
    from concourse.bass_utils import run_bass_kernel_spmd
    import ml_dtypes

    bf = ml_dtypes.bfloat16
    h = np.ascontiguousarray(np.asarray(h, dtype=np.float32))
    W = np.asarray(W_self, dtype=np.float32)
    src = np.asarray(src, dtype=np.int64)
    dst = np.asarray(dst, dtype=np.int64)

    order = np.argsort(dst, kind="stable")
    src_s = src[order]
    dst_s = dst[order]

    # per-(core, tile) edge ranges; tiles are 128 consecutive owned nodes
    tile_base = []
    for c in range(CORES):
        for t in range(NT):
            tile_base.append(c * NPC + t * 128)
    bounds_lo = np.searchsorted(dst_s, np.array(tile_base), side="left")
    hi_nodes = [min(b + 128, (b // NPC + 1) * NPC) for b in tile_base]
    bounds_hi = np.searchsorted(dst_s, np.array(hi_nodes), side="left")

    # split each tile's edges by src block; caps shared across cores (SPMD)
    per_ct = {}
    cnt = np.zeros((CORES, NT, NPB), dtype=np.int64)
    for c in range(CORES):
        for t in range(NT):
            i = c * NT + t
            lo, hi = int(bounds_lo[i]), int(bounds_hi[i])
            blk = (src_s[lo:hi] >= NBH0).astype(np.int64)
            for s_ in range(NPB):
                sel = np.nonzero(blk == s_)[0]
                per_ct[(c, t, s_)] = (src_s[lo:hi][sel], dst_s[lo:hi][sel] - tile_base[i])
                cnt[c, t, s_] = len(sel)
    caps = [
        [int((cnt[:, t, s_].max() + 127) // 128) for s_ in range(NPB)]
        for t in range(NT)
    ]
    NCH = sum(sum(r) for r in caps)

    # bf16 inputs: hT (transposed, k-blocked per tile), hrow, WT
    hp = np.zeros((NPAD, D), dtype=bf)
    hp[:N_NODES] = h.astype(bf)
    hT = np.ascontiguousarray(
        hp.reshape(NT_ALL, 128, 2, 128).transpose(3, 0, 2, 1)
    )  # [p, i, kb, m] = h[i*128+m, kb*128+p]
    WT = np.ascontiguousarray(
        W.T.astype(bf).reshape(2, 128, D).transpose(1, 0, 2)
    )  # [p, kb, j] = W[j, kb*128+p]
    iota = np.ascontiguousarray(
        np.broadcast_to(np.arange(128, dtype=np.float32), (128, 128)).astype(bf)
    )

    # region order: all (t, s=0) then all (t, s=1)
    regions = [(t, 0) for t in range(NT)] + [(t, 1) for t in range(NT)]

    in_maps = []
    for c in range(CORES):
        idx_parts = []
        dstl_all = np.full((128, NCH), -1.0, dtype=np.float32)
        coff = 0
        for t, s_ in regions:
            Cs = caps[t][s_]
            CAPs = 128 * Cs
            ss, dl = per_ct[(c, t, s_)]
            n = len(ss)
            spad = np.zeros(CAPs, dtype=np.int64)
            spad[:n] = ss - s_ * NBH0      # block-local row index
            idx_parts.append(_wrap_idx(spad))
            ei = np.arange(n)
            dstl_all[ei % 128, coff + ei // 128] = dl
            coff += Cs
        in_maps.append({
            "hT": hT,
            "hrow": hp,
            "WT": WT,
            "iota": iota,
            "dstl": dstl_all,
            "idx": np.ascontiguousarray(np.concatenate(idx_parts, axis=1)),
        })

    key = tuple(tuple(r) for r in caps)
    if key not in _cache:
        _cache[key] = _build(caps)
    nc = _cache[key]

    res = run_bass_kernel_spmd(nc, in_maps, core_ids=list(range(CORES)))
    out = np.concatenate(
        [res.results[c]["out"][:NPC] for c in range(CORES)], axis=0
    ).astype(np.float32)

    # zero-in-degree nodes keep h (host-side fixup)
    deg = np.bincount(dst.astype(np.int64), minlength=N_NODES)
    out[deg == 0] = h[deg == 0]
    return out


# revision 6
# speedup vs baseline: 1.5633x; 1.3673x over previous
"""DeepSATConv v3: T=256 dst tiles + per-region src dedup + host-built S.

Same math as v2 (kernel.py).  Differences:
  - dst tiles of 256 nodes (NT=10): acc spans 2 PSUM banks, 2 selector
    matmuls per chunk (S halves as stationary operands).
  - per (tile, src-block) the edge list is deduplicated by src: one
    gathered slot serves up to 2 edges (distinct dsts) of the same src;
    a src with k edges uses ceil(k/2) slots.  This cuts Q7 descriptor
    generation (the serial bottleneck at ~7ns/slot) by ~10-15%.
  - S is no longer one-hot-by-construction, so it is built on host
    (bf16 counts, mostly 0/1, sometimes 2) and DMA'd per tile.
"""

import numpy as np

N_NODES = 20000
N_EDGES = 320000
D = 256
CORES = 8
NPC = N_NODES // CORES          # 2500 nodes per core
TS = 256                        # dst tile size
NT = (NPC + TS - 1) // TS       # 10 dst tiles per core
NROWS = NT * TS                 # 2560 padded rows per core
NT_ALL = 160                    # phase-A tiles
NPAD = NT_ALL * 128             # 20480
NPB = 2
NT_B0 = 40                      # phase-A tiles in src block 0 (25%)
NBH0 = NT_B0 * 128              # 6144 rows
NBH1 = NPAD - NBH0              # 14336 rows

_cache = {}


def _build(caps):
    import concourse.bacc as bacc
    import concourse.mybir as mybir
    from concourse.tile import TileContext

    nc = bacc.Bacc("TRN2")
    f32 = mybir.dt.float32
    bf16 = mybir.dt.bfloat16

    NCH = sum(sum(r) for r in caps)     # total chunks across tiles/blocks
    NIX = 128 * NCH                     # total gathered slots
    CMAX = [max(c[s] for c in caps) for s in range(NPB)]
    CTMAX = max(c[0] + c[1] for c in caps)

    hT_d = nc.dram_tensor("hT", [128, NT_ALL, 2, 128], bf16, kind="ExternalInput")
    hrow_d = nc.dram_tensor("hrow", [128, NT_ALL, D], bf16, kind="ExternalInput")
    WT_d = nc.dram_tensor("WT", [128, 2, D], bf16, kind="ExternalInput")
    S_d = nc.dram_tensor("S", [128, NCH, TS], bf16, kind="ExternalInput")
    idx_d = nc.dram_tensor("idx", [128, NIX // 16], mybir.dt.int16, kind="ExternalInput")
    out_d = nc.dram_tensor("out", [NROWS, D], f32, kind="ExternalOutput")

    with TileContext(nc) as tc:
        with (
            tc.tile_pool(name="const", bufs=1) as constp,
            tc.tile_pool(name="pha", bufs=3) as pha,
            tc.tile_pool(name="gat0", bufs=NT) as gat0,
            tc.tile_pool(name="gat1", bufs=2) as gat1,
            tc.tile_pool(name="swk", bufs=2) as swk,
            tc.tile_pool(name="fin", bufs=2) as fin,
            tc.tile_pool(name="psa", bufs=2, space="PSUM") as psa,
            tc.tile_pool(name="psb", bufs=2, space="PSUM") as psb,
            tc.tile_pool(name="dram", bufs=1, space="DRAM") as dramp,
        ):
            z_blk = []
            for s_ in range(NPB):
                zb = dramp.tile([NBH0 if s_ == 0 else NBH1, 2 * D], bf16, tag=f"zblk{s_}")
                z_blk.append(zb)

            WT_sb = constp.tile([128, 2, D], bf16)
            nc.sync.dma_start(WT_sb[:, :, :], WT_d[:, :, :])
            idx_sb = constp.tile([128, NIX // 16], mybir.dt.int16)
            nc.sync.dma_start(idx_sb[:, :], idx_d[:, :])

            # ---- phase A: Z = [exp(h @ W.T) | exp(h @ W.T) * h], all nodes ----
            # batches of G=4 node-tiles: one DMA / matmul-chain / exp / mult
            # per group to amortize DMA fixed latency and instruction overhead
            G = 4
            for g in range(NT_ALL // G):
                i0 = g * G
                hT_sb = pha.tile([128, G, 2, 128], bf16, tag="hT")
                nc.sync.dma_start(hT_sb[:, :, :, :], hT_d[:, i0:i0 + G, :, :])
                ps = psa.tile([128, G, D], f32, tag="ps")
                for u in range(G):
                    for kb in range(2):
                        nc.tensor.matmul(
                            ps[:, u, :], hT_sb[:, u, kb, :], WT_sb[:, kb, :],
                            start=(kb == 0), stop=(kb == 1),
                        )
                z_sb = pha.tile([128, G, 2 * D], bf16, tag="zs")
                nc.scalar.activation(
                    z_sb[:, :, 0:D], ps[:, :, :], mybir.ActivationFunctionType.Exp
                )
                hr_sb = pha.tile([128, G, D], bf16, tag="hr")
                nc.sync.dma_start(hr_sb[:, :, :], hrow_d[:, i0:i0 + G, :])
                nc.vector.tensor_tensor(
                    z_sb[:, :, D:2 * D], z_sb[:, :, 0:D], hr_sb[:, :, :],
                    mybir.AluOpType.mult,
                )
                for u in range(G):
                    i = i0 + u
                    if i < NT_B0:
                        nc.sync.dma_start(z_blk[0][i * 128:(i + 1) * 128, :], z_sb[:, u, :])
                    else:
                        row = (i - NT_B0) * 128
                        nc.sync.dma_start(z_blk[1][row:row + 128, :], z_sb[:, u, :])

            # chunk offsets in host layout: all (t, s=0) regions, then (t, s=1)
            off0 = [0] * NT
            off1 = [0] * NT
            o = 0
            for t in range(NT):
                off0[t] = o
                o += caps[t][0]
            for t in range(NT):
                off1[t] = o
                o += caps[t][1]

            # ---- phase B head: all block-0 gathers (Q7 stream never stalls) ----
            zx0 = []
            for t in range(NT):
                zt = gat0.tile([128, CMAX[0], 2 * D], bf16, tag="zx0")
                Cs = caps[t][0]
                io = off0[t] * 8
                nc.gpsimd.dma_gather(
                    zt[:, 0:Cs, :], z_blk[0][:, :],
                    idx_sb[:, io:io + 8 * Cs], 128 * Cs, 128 * Cs, 2 * D,
                    single_packet=False,
                )
                zx0.append(zt)

            # ---- phase B: per-tile block-1 gather, selector matmuls, finalize ----
            for t in range(NT):
                C0, C1 = caps[t][0], caps[t][1]
                zx1 = gat1.tile([128, CMAX[1], 2 * D], bf16, tag="zx1")
                io = off1[t] * 8
                nc.gpsimd.dma_gather(
                    zx1[:, 0:C1, :], z_blk[1][:, :],
                    idx_sb[:, io:io + 8 * C1], 128 * C1, 128 * C1, 2 * D,
                    single_packet=False,
                )
                S_sb = swk.tile([128, CTMAX, TS], bf16, tag="S")
                nc.sync.dma_start(S_sb[:, 0:C0, :], S_d[:, off0[t]:off0[t] + C0, :])
                nc.sync.dma_start(S_sb[:, C0:C0 + C1, :], S_d[:, off1[t]:off1[t] + C1, :])
                acc0 = psb.tile([128, 2 * D], f32, tag="acc0")
                acc1 = psb.tile([128, 2 * D], f32, tag="acc1")
                for j in range(C0 + C1):
                    src_t = zx0[t] if j < C0 else zx1
                    jj = j if j < C0 else j - C0
                    nc.tensor.matmul(
                        acc0[:, :], S_sb[:, j, 0:128], src_t[:, jj, :],
                        start=(j == 0), stop=(j == C0 + C1 - 1),
                    )
                    nc.tensor.matmul(
                        acc1[:, :], S_sb[:, j, 128:256], src_t[:, jj, :],
                        start=(j == 0), stop=(j == C0 + C1 - 1),
                    )

                # ---- finalize tile (two 128-dst halves) ----
                for half, acc in ((0, acc0), (1, acc1)):
                    accs = fin.tile([128, 2 * D], f32, tag="accs")
                    nc.scalar.copy(accs[:, :], acc[:, :])
                    dmax = fin.tile([128, D], f32, tag="dmax")
                    nc.vector.tensor_scalar(
                        dmax[:, :], accs[:, 0:D], 1e-37, None, mybir.AluOpType.max
                    )
                    nc.vector.reciprocal(dmax[:, :], dmax[:, :])
                    nc.vector.tensor_tensor(
                        accs[:, 0:D], accs[:, D:2 * D], dmax[:, :],
                        mybir.AluOpType.mult,
                    )
                    ro = t * TS + half * 128
                    nc.sync.dma_start(out_d[ro:ro + 128, :], accs[:, 0:D])
    nc.compile()
    return nc


def _wrap_idx(ix):
    w = ix.astype(np.int16).reshape(-1, 8, 16).transpose(2, 0, 1).reshape(16, -1)
    return np.tile(w, (8, 1))


def _dedup_slots(ss, dl):
    """Slots of (src, [dsts]) with <=2 edges per slot, same src per slot.

    Returns (slot_src, slot_of_edge) with edges in (ss, dl) order.
    """
    n = len(ss)
    if n == 0:
        return np.zeros(0, dtype=np.int64), np.zeros(0, dtype=np.int64)
    order = np.argsort(ss, kind="stable")
    inv_order = np.empty(n, dtype=np.int64)
    inv_order[order] = np.arange(n)
    ss_s = ss[order]
    u, first = np.unique(ss_s, return_index=True)
    cnts = np.diff(np.r_[first, n])
    starts = np.r_[0, np.cumsum(cnts)[:-1]]
    grp = np.repeat(np.arange(len(u)), cnts)
    rank = np.arange(n) - starts[grp]
    slots_per = (cnts + 1) // 2
    slot_base = np.r_[0, np.cumsum(slots_per)[:-1]]
    slot_sorted = slot_base[grp] + rank // 2
    slot_src = np.repeat(u, slots_per)
    return slot_src, slot_sorted[inv_order]


def kernel(h, W_nb, b_nb, W_self, b_self, src, dst):
    from concourse.bass_utils import run_bass_kernel_spmd
    import ml_dtypes

    bf = ml_dtypes.bfloat16
    h = np.ascontiguousarray(np.asarray(h, dtype=np.float32))
    W = np.asarray(W_self, dtype=np.float32)
    src = np.asarray(src, dtype=np.int64)
    dst = np.asarray(dst, dtype=np.int64)

    order = np.argsort(dst, kind="stable")
    src_s = src[order]
    dst_s = dst[order]

    tile_base = []
    for c in range(CORES):
        for t in range(NT):
            tile_base.append(c * NPC + t * TS)
    bounds_lo = np.searchsorted(dst_s, np.array(tile_base), side="left")
    hi_nodes = [min(b + TS, (b // NPC + 1) * NPC) for b in tile_base]
    bounds_hi = np.searchsorted(dst_s, np.array(hi_nodes), side="left")

    # dedup slots per (core, tile, src-block)
    per_ct = {}
    cnt = np.zeros((CORES, NT, NPB), dtype=np.int64)
    for c in range(CORES):
        for t in range(NT):
            i = c * NT + t
            lo, hi = int(bounds_lo[i]), int(bounds_hi[i])
            e_src = src_s[lo:hi]
            e_dst = dst_s[lo:hi] - tile_base[i]
            blk = (e_src >= NBH0).astype(np.int64)
            for s_ in range(NPB):
                sel = np.nonzero(blk == s_)[0]
                slot_src, slot_of_edge = _dedup_slots(e_src[sel], e_dst[sel])
                per_ct[(c, t, s_)] = (slot_src, slot_of_edge, e_dst[sel])
                cnt[c, t, s_] = len(slot_src)
    caps = [
        [int((cnt[:, t, s_].max() + 127) // 128) for s_ in range(NPB)]
        for t in range(NT)
    ]
    NCH = sum(sum(r) for r in caps)

    hp = np.zeros((NPAD, D), dtype=bf)
    hp[:N_NODES] = h.astype(bf)
    hT = np.ascontiguousarray(
        hp.reshape(NT_ALL, 128, 2, 128).transpose(3, 0, 2, 1)
    )
    hrow = np.ascontiguousarray(hp.reshape(NT_ALL, 128, D).transpose(1, 0, 2))
    WT = np.ascontiguousarray(
        W.T.astype(bf).reshape(2, 128, D).transpose(1, 0, 2)
    )

    regions = [(t, 0) for t in range(NT)] + [(t, 1) for t in range(NT)]

    in_maps = []
    for c in range(CORES):
        idx_parts = []
        S_all = np.zeros((128, NCH, TS), dtype=np.float32)
        coff = 0
        for t, s_ in regions:
            Cs = caps[t][s_]
            CAPs = 128 * Cs
            slot_src, slot_of_edge, e_dst = per_ct[(c, t, s_)]
            n = len(slot_src)
            spad = np.zeros(CAPs, dtype=np.int64)
            spad[:n] = slot_src - s_ * NBH0
            idx_parts.append(_wrap_idx(spad))
            np.add.at(
                S_all,
                (slot_of_edge % 128, coff + slot_of_edge // 128, e_dst),
                1.0,
            )
            coff += Cs
        in_maps.append({
            "hT": hT,
            "hrow": hrow,
            "WT": WT,
            "S": S_all.astype(bf),
            "idx": np.ascontiguousarray(np.concatenate(idx_parts, axis=1)),
        })

    key = tuple(tuple(r) for r in caps)
    if key not in _cache:
        _cache[key] = _build(caps)
    nc = _cache[key]

    res = run_bass_kernel_spmd(nc, in_maps, core_ids=list(range(CORES)))
    out = np.concatenate(
        [res.results[c]["out"][:NPC] for c in range(CORES)], axis=0
    ).astype(np.float32)

    deg = np.bincount(dst.astype(np.int64), minlength=N_NODES)
    out[deg == 0] = h[deg == 0]
    return out


# revision 8
# speedup vs baseline: 1.6361x; 1.0466x over previous
"""DeepSATConv v3: T=256 dst tiles + per-region src dedup + host-built S.

Same math as v2 (kernel.py).  Differences:
  - dst tiles of 256 nodes (NT=10): acc spans 2 PSUM banks, 2 selector
    matmuls per chunk (S halves as stationary operands).
  - per (tile, src-block) the edge list is deduplicated by src: one
    gathered slot serves up to 2 edges (distinct dsts) of the same src;
    a src with k edges uses ceil(k/2) slots.  This cuts Q7 descriptor
    generation (the serial bottleneck at ~7ns/slot) by ~10-15%.
  - S is no longer one-hot-by-construction, so it is built on host
    (bf16 counts, mostly 0/1, sometimes 2) and DMA'd per tile.
"""

import numpy as np

N_NODES = 20000
N_EDGES = 320000
D = 256
CORES = 8
NPC = N_NODES // CORES          # 2500 nodes per core
TS = 256                        # dst tile size
NT = (NPC + TS - 1) // TS       # 10 dst tiles per core
NROWS = NT * TS                 # 2560 padded rows per core
NT_ALL = 160                    # phase-A tiles
NPAD = NT_ALL * 128             # 20480
NPB = 2
NT_B0 = 40                      # phase-A tiles in src block 0 (25%)
NBH0 = NT_B0 * 128              # 6144 rows
NBH1 = NPAD - NBH0              # 14336 rows

_cache = {}


def _build(caps):
    import concourse.bacc as bacc
    import concourse.mybir as mybir
    from concourse.tile import TileContext

    nc = bacc.Bacc("TRN2")
    f32 = mybir.dt.float32
    bf16 = mybir.dt.bfloat16

    NCH = sum(sum(r) for r in caps)     # total chunks across tiles/blocks
    NIX = 128 * NCH                     # total gathered slots
    CMAX = [max(c[s] for c in caps) for s in range(NPB)]
    CTMAX = max(c[0] + c[1] for c in caps)

    hT_d = nc.dram_tensor("hT", [128, NT_ALL, 2, 128], bf16, kind="ExternalInput")
    hrow_d = nc.dram_tensor("hrow", [128, NT_ALL, D], bf16, kind="ExternalInput")
    WT_d = nc.dram_tensor("WT", [128, 2, D], bf16, kind="ExternalInput")
    S_d = nc.dram_tensor("S", [128, NCH, TS], bf16, kind="ExternalInput")
    idx_d = nc.dram_tensor("idx", [128, NIX // 16], mybir.dt.int16, kind="ExternalInput")
    out_d = nc.dram_tensor("out", [NROWS, D], f32, kind="ExternalOutput")

    with TileContext(nc) as tc:
        with (
            tc.tile_pool(name="const", bufs=1) as constp,
            tc.tile_pool(name="pha", bufs=4) as pha,
            tc.tile_pool(name="gat0", bufs=NT) as gat0,
            tc.tile_pool(name="gat1", bufs=2) as gat1,
            tc.tile_pool(name="swk", bufs=2) as swk,
            tc.tile_pool(name="fin", bufs=2) as fin,
            tc.tile_pool(name="psa", bufs=2, space="PSUM") as psa,
            tc.tile_pool(name="psb", bufs=2, space="PSUM") as psb,
            tc.tile_pool(name="dram", bufs=1, space="DRAM") as dramp,
        ):
            z_blk = []
            for s_ in range(NPB):
                zb = dramp.tile(
                    [128, (NBH0 if s_ == 0 else NBH1) // 128, 2 * D], bf16,
                    tag=f"zblk{s_}",
                )
                z_blk.append(zb)

            WT_sb = constp.tile([128, 2, D], bf16)
            nc.sync.dma_start(WT_sb[:, :, :], WT_d[:, :, :])
            idx_sb = constp.tile([128, NIX // 16], mybir.dt.int16)
            nc.sync.dma_start(idx_sb[:, :], idx_d[:, :])

            # ---- phase A: Z = [exp(h @ W.T) | exp(h @ W.T) * h], all nodes ----
            # batches of G=4 node-tiles: one DMA / matmul-chain / exp / mult
            # per group to amortize DMA fixed latency and instruction overhead
            G = 4
            for g in range(NT_ALL // G):
                i0 = g * G
                hT_sb = pha.tile([128, G, 2, 128], bf16, tag="hT")
                nc.sync.dma_start(hT_sb[:, :, :, :], hT_d[:, i0:i0 + G, :, :])
                ps = psa.tile([128, G, D], f32, tag="ps")
                for u in range(G):
                    for kb in range(2):
                        nc.tensor.matmul(
                            ps[:, u, :], hT_sb[:, u, kb, :], WT_sb[:, kb, :],
                            start=(kb == 0), stop=(kb == 1),
                        )
                z_sb = pha.tile([128, G, 2 * D], bf16, tag="zs")
                nc.scalar.activation(
                    z_sb[:, :, 0:D], ps[:, :, :], mybir.ActivationFunctionType.Exp
                )
                hr_sb = pha.tile([128, G, D], bf16, tag="hr")
                nc.sync.dma_start(hr_sb[:, :, :], hrow_d[:, i0:i0 + G, :])
                nc.vector.tensor_tensor(
                    z_sb[:, :, D:2 * D], z_sb[:, :, 0:D], hr_sb[:, :, :],
                    mybir.AluOpType.mult,
                )
                if i0 < NT_B0:
                    nc.sync.dma_start(z_blk[0][:, i0:i0 + G, :], z_sb[:, :, :])
                else:
                    li = i0 - NT_B0
                    nc.sync.dma_start(z_blk[1][:, li:li + G, :], z_sb[:, :, :])

            # chunk offsets in host layout: all (t, s=0) regions, then (t, s=1)
            off0 = [0] * NT
            off1 = [0] * NT
            o = 0
            for t in range(NT):
                off0[t] = o
                o += caps[t][0]
            for t in range(NT):
                off1[t] = o
                o += caps[t][1]

            # ---- phase B head: all block-0 gathers (Q7 stream never stalls) ----
            zx0 = []
            for t in range(NT):
                zt = gat0.tile([128, CMAX[0], 2 * D], bf16, tag="zx0")
                Cs = caps[t][0]
                io = off0[t] * 8
                nc.gpsimd.dma_gather(
                    zt[:, 0:Cs, :], z_blk[0][:, :, :].flatten_outer_dims(),
                    idx_sb[:, io:io + 8 * Cs], 128 * Cs, 128 * Cs, 2 * D,
                    single_packet=False,
                )
                zx0.append(zt)

            # ---- phase B: per-tile block-1 gather, selector matmuls, finalize ----
            for t in range(NT):
                C0, C1 = caps[t][0], caps[t][1]
                zx1 = gat1.tile([128, CMAX[1], 2 * D], bf16, tag="zx1")
                io = off1[t] * 8
                nc.gpsimd.dma_gather(
                    zx1[:, 0:C1, :], z_blk[1][:, :, :].flatten_outer_dims(),
                    idx_sb[:, io:io + 8 * C1], 128 * C1, 128 * C1, 2 * D,
                    single_packet=False,
                )
                S_sb = swk.tile([128, CTMAX, TS], bf16, tag="S")
                nc.sync.dma_start(S_sb[:, 0:C0, :], S_d[:, off0[t]:off0[t] + C0, :])
                nc.sync.dma_start(S_sb[:, C0:C0 + C1, :], S_d[:, off1[t]:off1[t] + C1, :])
                acc0 = psb.tile([128, 2 * D], f32, tag="acc0")
                acc1 = psb.tile([128, 2 * D], f32, tag="acc1")
                for j in range(C0 + C1):
                    src_t = zx0[t] if j < C0 else zx1
                    jj = j if j < C0 else j - C0
                    nc.tensor.matmul(
                        acc0[:, :], S_sb[:, j, 0:128], src_t[:, jj, :],
                        start=(j == 0), stop=(j == C0 + C1 - 1),
                    )
                    nc.tensor.matmul(
                        acc1[:, :], S_sb[:, j, 128:256], src_t[:, jj, :],
                        start=(j == 0), stop=(j == C0 + C1 - 1),
                    )

                # ---- finalize tile (two 128-dst halves) ----
                for half, acc in ((0, acc0), (1, acc1)):
                    rec = fin.tile([128, D], f32, tag="rec")
                    nc.vector.reciprocal(rec[:, :], acc[:, 0:D])
                    res = fin.tile([128, D], f32, tag="res")
                    nc.vector.tensor_tensor(
                        res[:, :], acc[:, D:2 * D], rec[:, :],
                        mybir.AluOpType.mult,
                    )
                    ro = t * TS + half * 128
                    nc.sync.dma_start(out_d[ro:ro + 128, :], res[:, :])
    nc.compile()
    return nc


def _wrap_idx(ix):
    w = ix.astype(np.int16).reshape(-1, 8, 16).transpose(2, 0, 1).reshape(16, -1)
    return np.tile(w, (8, 1))


def _dedup_slots(ss, dl):
    """Slots of (src, [dsts]) with <=2 edges per slot, same src per slot.

    Returns (slot_src, slot_of_edge) with edges in (ss, dl) order.
    """
    n = len(ss)
    if n == 0:
        return np.zeros(0, dtype=np.int64), np.zeros(0, dtype=np.int64)
    u, inv = np.unique(ss, return_inverse=True)
    return u, inv


def kernel(h, W_nb, b_nb, W_self, b_self, src, dst):
    from concourse.bass_utils import run_bass_kernel_spmd
    import ml_dtypes

    bf = ml_dtypes.bfloat16
    h = np.ascontiguousarray(np.asarray(h, dtype=np.float32))
    W = np.asarray(W_self, dtype=np.float32)
    src = np.asarray(src, dtype=np.int64)
    dst = np.asarray(dst, dtype=np.int64)

    order = np.argsort(dst, kind="stable")
    src_s = src[order]
    dst_s = dst[order]

    tile_base = []
    for c in range(CORES):
        for t in range(NT):
            tile_base.append(c * NPC + t * TS)
    bounds_lo = np.searchsorted(dst_s, np.array(tile_base), side="left")
    hi_nodes = [min(b + TS, (b // NPC + 1) * NPC) for b in tile_base]
    bounds_hi = np.searchsorted(dst_s, np.array(hi_nodes), side="left")

    # dedup slots per (core, tile, src-block)
    per_ct = {}
    cnt = np.zeros((CORES, NT, NPB), dtype=np.int64)
    for c in range(CORES):
        for t in range(NT):
            i = c * NT + t
            lo, hi = int(bounds_lo[i]), int(bounds_hi[i])
            e_src = src_s[lo:hi]
            e_dst = dst_s[lo:hi] - tile_base[i]
            blk = (e_src >= NBH0).astype(np.int64)
            for s_ in range(NPB):
                sel = np.nonzero(blk == s_)[0]
                slot_src, slot_of_edge = _dedup_slots(e_src[sel], e_dst[sel])
                per_ct[(c, t, s_)] = (slot_src, slot_of_edge, e_dst[sel])
                cnt[c, t, s_] = len(slot_src)
    caps = [
        [int((cnt[:, t, s_].max() + 127) // 128) for s_ in range(NPB)]
        for t in range(NT)
    ]
    NCH = sum(sum(r) for r in caps)

    hp = np.zeros((NPAD, D), dtype=bf)
    hp[:N_NODES] = h.astype(bf)
    hT = np.ascontiguousarray(
        hp.reshape(NT_ALL, 128, 2, 128).transpose(3, 0, 2, 1)
    )
    hrow = np.ascontiguousarray(hp.reshape(NT_ALL, 128, D).transpose(1, 0, 2))
    WT = np.ascontiguousarray(
        W.T.astype(bf).reshape(2, 128, D).transpose(1, 0, 2)
    )

    regions = [(t, 0) for t in range(NT)] + [(t, 1) for t in range(NT)]

    in_maps = []
    for c in range(CORES):
        idx_parts = []
        S_all = np.zeros((128, NCH, TS), dtype=np.float32)
        coff = 0
        for t, s_ in regions:
            Cs = caps[t][s_]
            CAPs = 128 * Cs
            slot_src, slot_of_edge, e_dst = per_ct[(c, t, s_)]
            n = len(slot_src)
            spad = np.zeros(CAPs, dtype=np.int64)
            bl = slot_src - s_ * NBH0      # block-local node index
            K = (NBH0 if s_ == 0 else NBH1) // 128
            spad[:n] = (bl % 128) * K + bl // 128   # p-major row
            idx_parts.append(_wrap_idx(spad))
            np.add.at(
                S_all,
                (slot_of_edge % 128, coff + slot_of_edge // 128, e_dst),
                1.0,
            )
            coff += Cs
        in_maps.append({
            "hT": hT,
            "hrow": hrow,
            "WT": WT,
            "S": S_all.astype(bf),
            "idx": np.ascontiguousarray(np.concatenate(idx_parts, axis=1)),
        })

    key = tuple(tuple(r) for r in caps)
    if key not in _cache:
        _cache[key] = _build(caps)
    nc = _cache[key]

    res = run_bass_kernel_spmd(nc, in_maps, core_ids=list(range(CORES)))
    out = np.concatenate(
        [res.results[c]["out"][:NPC] for c in range(CORES)], axis=0
    ).astype(np.float32)

    deg = np.bincount(dst.astype(np.int64), minlength=N_NODES)
    out[deg == 0] = h[deg == 0]
    return out
